# revision 23
# baseline (speedup 1.0000x reference)
"""Trainium2 Bass kernel for nn_ConditionalSDFVAE (MoE routing VAE decoder).

Strategy: expert-parallel sharding with host-side token routing.
  - Tokens are grouped by label (the routing/shard step, done on host as part
    of sharding): core e receives exactly the tokens with label == e, padded
    to a multiple of 512.
  - The encoder affine z = mu_e + eps * exp(0.5*logvar_e) is folded into the
    first layer's weights on the host (constant O(E*L*H) preprocessing):
      h1 = relu(W0z.T z + W0x.T x + b0)
         = relu((W0z * sc).T eps + W0x.T x + (b0 + W0z.T mu))
    so the device consumes [eps; x] directly in a feature-major layout.
  - On device (per core): 3-layer expert MLP as a chain of matmuls
    ([features, tokens] layout, PSUM accumulate over K-chunks), ReLU+bias
    evacuations split across ScalarE/VectorE/GpSimd, final dot-product layer
    as an M=1 matmul.
  - Matmul precision modes: "split" (default) = bf16 hi/lo decomposition,
    3 matmul terms per logical matmul, ~1e-5 relative error at ~3x bf16
    cost; "bf16" = plain bf16 inputs (~6e-3 rel err, fastest); "f32" =
    native fp32 (exact, 4 cycles/row).
  - KL is a deterministic function of per-expert counts and the (E,L) tables,
    computed on host from the bincount already needed for routing.

Self-contained: hardcodes B=65536, D=3, L=64, H=256, E=8, n_hidden=2.
"""

import sys

for _p in ("/opt/trn_rl_repo",):
    if _p not in sys.path:
        sys.path.insert(0, _p)

import numpy as np

B, D, L, H, E = 65536, 3, 64, 256, 8
NHID = 2  # extra hidden layers
KIN = D + L  # 67
TFREE = 512  # token tile (matmul moving free dim)
MM_DT = "split"  # "split" | "bf16" | "f32" | "f32r"
CW = 1282  # packed weight tensor columns

_module_cache = {}


def _mdt(mybir, mm_dt):
    return {"f32r": mybir.dt.float32r,
            "bf16": mybir.dt.bfloat16,
            "split": mybir.dt.bfloat16,
            "f32": mybir.dt.float32}[mm_dt]


def _build_module(cap, mm_dt=MM_DT, cfg=None):
    """Build the SPMD Bass module for per-core capacity `cap` tokens."""
    import concourse.bass as bass
    import concourse.mybir as mybir
    import concourse.tile as tile
    from concourse import bacc

    f32 = mybir.dt.float32
    mdt = _mdt(mybir, mm_dt)
    AF = mybir.ActivationFunctionType
    ALU = mybir.AluOpType
    split = mm_dt == "split"

    cfg = dict(cfg or {})
    p1b = cfg.get("p1b", 3)
    pmb = cfg.get("pmb", 4)
    pob = cfg.get("pob", 1)
    hsb = cfg.get("hsb", 4)
    actb = cfg.get("actb", 3)
    sub_eng = cfg.get("sub_eng", "vector")  # engine for the lo-split subtract

    nt = cap // TFREE
    assert cap % TFREE == 0

    nc = bacc.Bacc("TRN2", target_bir_lowering=False, debug=False)

    # ext: rows 0..L-1 = eps(-scaled via weights), rows L..L+D-1 = x
    exts = [nc.dram_tensor(n, [KIN, cap], mdt, kind="ExternalInput").ap()
            for n in (("ext", "extl") if split else ("ext",))]
    # cw packs every matmul weight in one tensor:
    # cols 0..1023: wh[l][k] chunks (4 x 256), 1024..1279: w0p (rows 0..66),
    # cols 1280, 1281: wo chunks.
    cwds = [nc.dram_tensor(n, [128, CW], mdt, kind="ExternalInput").ap()
            for n in (("cw", "cwl") if split else ("cw",))]
    # cvec columns: 0..5 = (b0', bh0, bh1) x (chunk0, chunk1); col 6 row0 = bo
    cvec = nc.dram_tensor("cvec", [128, 7], f32, kind="ExternalInput").ap()
    outd = nc.dram_tensor("outv", [1, cap], f32, kind="ExternalOutput").ap()

    with tile.TileContext(nc) as tc:
        with (
            tc.tile_pool(name="consts", bufs=1) as consts,
            tc.tile_pool(name="acts", bufs=actb) as acts,
            tc.tile_pool(name="hs", bufs=hsb) as hs,
            tc.tile_pool(name="outs", bufs=1) as outs,
            tc.tile_pool(name="psum", bufs=1, space="PSUM") as pp,
        ):
            # ---- resident constants ----
            cws = []
            for i, cwd in enumerate(cwds):
                t_ = consts.tile([128, CW], mdt, name=f"cw{i}")
                nc.gpsimd.dma_start(out=t_, in_=cwd)
                cws.append(t_)
            cs = consts.tile([128, 7], f32)
            nc.gpsimd.dma_start(out=cs, in_=cvec)

            def wh_lhsT(w, l, k, m):
                c = (2 * l + k) * 256 + m * 128
                return w[:, c:c + 128]

            bts = [[cs[:, 2 * j + m: 2 * j + m + 1] for m in range(2)]
                   for j in range(1 + NHID)]
            bos = cs[0:1, 6:7]

            def mm_terms(wh_hi_ap, wh_lo_ap, rhs_hi, rhs_lo):
                """(lhsT, rhs) term list: hi*hi + hi*lo + lo*hi."""
                if split:
                    return [(wh_hi_ap, rhs_hi), (wh_hi_ap, rhs_lo),
                            (wh_lo_ap, rhs_hi)]
                return [(wh_hi_ap, rhs_hi)]

            def accum(pt, terms):
                n = len(terms)
                for i, (lh, rh) in enumerate(terms):
                    nc.tensor.matmul(pt, lhsT=lh, rhs=rh,
                                     start=(i == 0), stop=(i == n - 1))

            def evac_single(dst_pair, psum_ap, bias_ap, tag):
                hh = hs.tile([128, TFREE], mdt, tag=tag, name=tag)
                if tag.endswith("m0"):
                    nc.scalar.activation(hh, psum_ap, AF.Relu, bias=bias_ap)
                else:
                    nc.vector.tensor_scalar(hh, psum_ap, bias_ap, 0.0,
                                            op0=ALU.add, op1=ALU.max)
                dst_pair.append((hh, None))

            def split_pair(h32, tag):
                """h32 [128, 2T] f32 -> (hh, hl) [128, 2T] bf16 via one copy
                and one subtract; k-chunk slices come from the free dim."""
                hh = hs.tile([128, 2 * TFREE], mdt, tag=tag + "_h",
                             name=tag + "_h")
                nc.vector.tensor_copy(hh, h32)
                hl = hs.tile([128, 2 * TFREE], mdt, tag=tag + "_l",
                             name=tag + "_l")
                if sub_eng == "gpsimd":
                    nc.gpsimd.tensor_sub(hl, h32, hh)
                else:
                    nc.vector.tensor_sub(hl, h32, hh)
                return hh, hl

            # ---- token tiles ----
            repeat = cfg.get("repeat", 1)
            loop_ctx = tc.For_i(0, repeat, 1) if repeat > 1 else None
            if loop_ctx is not None:
                loop_ctx.__enter__()
            for t in range(nt):
                sl = bass.ts(t, TFREE)
                xins = []
                for i, extd in enumerate(exts):
                    xt_ = acts.tile([KIN, TFREE], mdt, tag=f"xin{i}",
                                    name=f"xin{i}")
                    nc.sync.dma_start(out=xt_, in_=extd[:, sl])
                    xins.append(xt_)
                xinh = xins[0]
                xinl = xins[1] if split else None

                # L0
                h1 = []
                if split:
                    h32 = hs.tile([128, 2 * TFREE], f32, tag="h1_32",
                                  name="h1_32")
                for m in range(2):
                    p = pp.tile([128, TFREE], f32, tag="p1", bufs=p1b)
                    w0h = cws[0][0:KIN, 1024 + m * 128:1024 + (m + 1) * 128]
                    w0l = (cws[1][0:KIN, 1024 + m * 128:1024 + (m + 1) * 128]
                           if split else None)
                    accum(p, mm_terms(w0h, w0l, xinh, xinl))
                    if split:
                        nc.scalar.activation(h32[:, bass.ts(m, TFREE)], p,
                                             AF.Relu, bias=bts[0][m])
                    else:
                        evac_single(h1, p, bts[0][m], f"h1m{m}")
                if split:
                    hh, hl = split_pair(h32, "h1")
                    h1 = [(hh[:, bass.ts(k, TFREE)], hl[:, bass.ts(k, TFREE)])
                          for k in range(2)]

                # hidden layers
                hp = h1
                for l in range(NHID):
                    hn = []
                    if split:
                        h32 = hs.tile([128, 2 * TFREE], f32, tag=f"h{l + 2}_32",
                                      name=f"h{l + 2}_32")
                    for m in range(2):
                        p = pp.tile([128, TFREE], f32, tag="pmid", bufs=pmb)
                        terms = []
                        for k in range(2):
                            terms += mm_terms(
                                wh_lhsT(cws[0], l, k, m),
                                wh_lhsT(cws[1], l, k, m) if split else None,
                                hp[k][0], hp[k][1])
                        accum(p, terms)
                        if split:
                            nc.scalar.activation(h32[:, bass.ts(m, TFREE)], p,
                                                 AF.Relu, bias=bts[1 + l][m])
                        else:
                            evac_single(hn, p, bts[1 + l][m], f"h{l + 2}m{m}")
                    if split:
                        hh, hl = split_pair(h32, f"h{l + 2}")
                        hn = [(hh[:, bass.ts(k, TFREE)],
                               hl[:, bass.ts(k, TFREE)]) for k in range(2)]
                    hp = hn

                # output layer: out = wo.T @ h + bo   (M=1)
                po = pp.tile([1, TFREE], f32, tag="po", bufs=pob)
                terms = []
                for k in range(2):
                    terms += mm_terms(cws[0][:, 1280 + k:1281 + k],
                                      cws[1][:, 1280 + k:1281 + k] if split
                                      else None,
                                      hp[k][0], hp[k][1])
                accum(po, terms)
                ot = outs.tile([1, TFREE], f32, tag="ot", bufs=3, name="ot")
                nc.vector.tensor_scalar(ot, po, bos, None, op0=ALU.add)
                nc.sync.dma_start(out=outd[:, sl], in_=ot)

            if loop_ctx is not None:
                loop_ctx.__exit__(None, None, None)

    nc.compile()
    return nc


def _get_module(cap, mm_dt=MM_DT, cfg=None):
    key = (cap, mm_dt, tuple(sorted((cfg or {}).items())))
    if key not in _module_cache:
        _module_cache[key] = _build_module(cap, mm_dt, cfg)
    return _module_cache[key]


def _np_mdt(mm_dt):
    if mm_dt in ("bf16", "split"):
        import ml_dtypes
        return ml_dtypes.bfloat16
    return np.float32


def _split_hi_lo(a, npdt):
    hi = a.astype(npdt)
    lo = (a - hi.astype(np.float32)).astype(npdt)
    return hi, lo


def _prepare(x, labels, eps, mu_table, logvar_table, W0, b0, Wh, bh, Wo, bo,
             mm_dt=MM_DT):
    """Host-side routing/sharding. Returns (in_maps, order, offs, counts, cap)."""
    npdt = _np_mdt(mm_dt)
    split = mm_dt == "split"
    x = np.ascontiguousarray(np.asarray(x, np.float32))
    eps = np.ascontiguousarray(np.asarray(eps, np.float32))
    lab = np.asarray(labels).astype(np.int64)
    order = np.argsort(lab, kind="stable")
    counts = np.bincount(lab, minlength=E)
    offs = np.concatenate([[0], np.cumsum(counts)])
    cap = int(max(TFREE, -(-counts.max() // TFREE) * TFREE))

    xs = x[order]
    es = eps[order]

    in_maps = []
    for e in range(E):
        seg = slice(int(offs[e]), int(offs[e + 1]))
        n_e = int(counts[e])
        ext = np.zeros((KIN, cap), np.float32)
        ext[:L, :n_e] = es[seg].T
        ext[L:, :n_e] = xs[seg].T
        sc = np.exp(0.5 * np.asarray(logvar_table[e], np.float32))  # [L]
        mu = np.asarray(mu_table[e], np.float32)
        w0z = np.asarray(W0[e, D:], np.float32)  # [L, H] (z rows)
        w0x = np.asarray(W0[e, :D], np.float32)  # [D, H]
        w0p = np.concatenate([w0z * sc[:, None], w0x], axis=0)  # [KIN, H]
        b0p = np.asarray(b0[e], np.float32) + mu @ w0z  # [H]
        cv = np.zeros((128, 7), np.float32)
        cv[:, 0] = b0p[:128]
        cv[:, 1] = b0p[128:]
        for l in range(NHID):
            bl = np.asarray(bh[l, e], np.float32)
            cv[:, 2 + 2 * l] = bl[:128]
            cv[:, 3 + 2 * l] = bl[128:]
        cv[0, 6] = np.float32(np.asarray(bo)[e, 0])
        # packed weight tensor (matches cw layout in _build_module)
        cwm = np.zeros((128, CW), np.float32)
        whe = np.asarray(Wh[:, e], np.float32)  # [NHID, H, H]
        for l in range(NHID):
            for k in range(2):
                cwm[:, (2 * l + k) * 256:(2 * l + k + 1) * 256] = \
                    whe[l, k * 128:(k + 1) * 128, :]
        cwm[:KIN, 1024:1280] = w0p
        woe = np.asarray(Wo[e], np.float32).reshape(H)
        cwm[:, 1280] = woe[:128]
        cwm[:, 1281] = woe[128:]
        m = {"cvec": cv}
        if split:
            m["ext"], m["extl"] = _split_hi_lo(ext, npdt)
            m["cw"], m["cwl"] = _split_hi_lo(cwm, npdt)
        else:
            m["ext"] = ext.astype(npdt)
            m["cw"] = cwm.astype(npdt)
        in_maps.append({k: np.ascontiguousarray(v) for k, v in m.items()})
    return in_maps, order, offs, counts, cap


def _kl_host(labels_counts, mu_table, logvar_table, batch):
    lv = np.asarray(logvar_table, np.float64)
    mu = np.asarray(mu_table, np.float64)
    s_e = (1.0 + lv - mu * mu - np.exp(lv)).sum(axis=1)
    return np.float32(-0.5 * float((labels_counts * s_e).sum()) / batch)


def kernel(x, labels, eps, mu_table, logvar_table, W0, b0, Wh, bh, Wo, bo,
           _trace=False):
    from concourse import bass_utils

    in_maps, order, offs, counts, cap = _prepare(
        x, labels, eps, mu_table, logvar_table, W0, b0, Wh, bh, Wo, bo)
    nc = _get_module(cap)
    try:
        res = bass_utils.run_bass_kernel_spmd(nc, in_maps,
                                              core_ids=list(range(E)),
                                              trace=_trace)
    except Exception:
        # transient device failures (e.g. NRT unrecoverable) usually clear
        # on a fresh session; retry once
        res = bass_utils.run_bass_kernel_spmd(nc, in_maps,
                                              core_ids=list(range(E)),
                                              trace=_trace)

    batch = x.shape[0]
    outv = np.zeros((batch,), np.float32)
    for e in range(E):
        n_e = int(counts[e])
        if n_e:
            outv[order[int(offs[e]):int(offs[e + 1])]] = \
                res.results[e]["outv"][0, :n_e]
    out = outv[:, None].astype(np.float32)
    kl = _kl_host(counts, mu_table, logvar_table, batch)
    if _trace:
        kernel._last_results = res
    return out, kl
